# revision 52
# baseline (speedup 1.0000x reference)
"""Gemma3 sliding-window attention on 8 Trainium2 NeuronCores.

Sharding: 8 cores = 2 batches x 4 KV-head groups (2 query heads / 1 KV head
per core).  Per core: QKV projections (column-sharded), fused RMSNorm+RoPE
(q/k scales folded into host-built RoPE tables), sliding-window attention in
transposed score layout (S^T = K^T q), row-sharded Wo with a 4-way
ReduceScatter per 512-token group.  bf16 matmuls, fp32 PSUM accumulation.
"""

import os
import sys
import types

import numpy as np

B, T, H = 2, 2048, 2560
NH, NKV, D = 8, 4, 256
WINDOW = 1024
EPS = 1e-6
ROPE_THETA = 10000.0

N_CORES = 8
NHC = NH // NKV      # query heads per core (2)
NDC = NHC * D        # head dims per core (512)
TB = T // 128        # 16 token blocks
HB = H // 128        # 20 hidden chunks
NQG = T // 512       # 4 query groups
RS_SPLITS = [2, 2, 2, 2]   # ReduceScatter chunks per query group
# mask tile index by delta = kb*128 - qg*512
MASK_DELTAS = [0, 128, 256, 384, -640, -768, -896, -1024]

_TRN_ROOT_CANDIDATES = ("/opt/trn_rl_repo", "/root/.axon_site/_ro/trn_rl_repo")
_PROG_CACHE = {}
LAST_RESULTS = None


def _ensure_import_paths():
    for p in _TRN_ROOT_CANDIDATES:
        if os.path.isdir(p) and p not in sys.path:
            sys.path.insert(0, p)
            break


def _install_ntff_hook():
    """Register the axon NTFF profiling hook (missing antenv.axon_hooks shim)."""
    modname = "antenv.axon_hooks"
    if modname in sys.modules:
        return
    try:
        import antenv
        mod = types.ModuleType(modname)
        holder = {}
        mod.set_axon_ntff_profile_hook = lambda h: holder.__setitem__("h", h)
        mod.get_axon_ntff_profile_hook = lambda: holder.get("h")
        sys.modules[modname] = mod
        antenv.axon_hooks = mod
        from trn_agent_boot.trn_boot import _ntff_profile_via_ctypes
        mod.set_axon_ntff_profile_hook(
            _ntff_profile_via_ctypes("/opt/axon/libaxon_pjrt.so")
        )
    except Exception:
        pass


def _build_program():
    if "prog" in _PROG_CACHE:
        return _PROG_CACHE["prog"]

    _ensure_import_paths()
    import concourse.bass as bass
    import concourse.tile as tile
    from concourse import bacc, mybir

    dt = mybir.dt
    f32, bf16 = dt.float32, dt.bfloat16
    AF = mybir.ActivationFunctionType
    ALU = mybir.AluOpType
    PSUM = bass.MemorySpace.PSUM

    nc = bacc.Bacc("TRN2", target_bir_lowering=False, debug=False,
                   num_devices=N_CORES)

    # ---------------- I/O ----------------
    x_d = nc.dram_tensor("x", [T, H], bf16, kind="ExternalInput")
    wq_d = nc.dram_tensor("wq", [H, NDC], bf16, kind="ExternalInput")
    wkv_d = nc.dram_tensor("wkv", [H, 2 * D], bf16, kind="ExternalInput")
    wo_d = nc.dram_tensor("wo", [NDC, H], bf16, kind="ExternalInput")
    tab_names = ("cq1", "sq1", "cq2", "sq2", "ck1", "sk1", "ck2", "sk2")
    tabs_d = nc.dram_tensor("tabs", [T, len(tab_names) * (D // 2)], f32,
                            kind="ExternalInput")
    masks_d = nc.dram_tensor("masks", [len(MASK_DELTAS), 128, 512], bf16,
                             kind="ExternalInput")
    ident_d = nc.dram_tensor("ident", [128, 128], bf16, kind="ExternalInput")
    identf_d = nc.dram_tensor("identf", [128, 128], f32, kind="ExternalInput")
    y_outs = [nc.dram_tensor(f"y{j}", [128, H], bf16, kind="ExternalOutput")
              for j in range(NQG)]

    mask_idx = {d: i for i, d in enumerate(MASK_DELTAS)}

    with tile.TileContext(nc) as tc:
        import contextlib
        with contextlib.ExitStack() as ctx:
            const = ctx.enter_context(tc.tile_pool(name="const", bufs=1))
            # PSUM: tag "acc" (accumulators) 4 banks + tag "mm" (stream) 4
            pacc = ctx.enter_context(
                tc.tile_pool(name="pacc", bufs=8, space="PSUM"))
            pmm = pacc
            xnp = ctx.enter_context(tc.tile_pool(name="xnp", bufs=2))
            xtp = ctx.enter_context(tc.tile_pool(name="xtp", bufs=7))
            tabp = ctx.enter_context(tc.tile_pool(name="tabp", bufs=2))
            qnp = ctx.enter_context(tc.tile_pool(name="qnp", bufs=4))
            scrp = ctx.enter_context(tc.tile_pool(name="scrp", bufs=4))
            statp = ctx.enter_context(tc.tile_pool(name="statp", bufs=8))
            ropep = ctx.enter_context(tc.tile_pool(name="ropep", bufs=8))
            qTp = ctx.enter_context(tc.tile_pool(name="qTp", bufs=4))
            pTp = ctx.enter_context(tc.tile_pool(name="pTp", bufs=4))
            smp = ctx.enter_context(tc.tile_pool(name="smp", bufs=2))
            ybp = ctx.enter_context(tc.tile_pool(name="ybp", bufs=2))
            dramp = ctx.enter_context(
                tc.tile_pool(name="dramp", bufs=1, space="DRAM"))

            # -------- resident constants --------
            # first x blocks before the bulky weight loads so PE can start
            xn_pre = {}
            for tb0 in range(2):
                xn = xnp.tile([128, H], bf16, name="xn")
                nc.sync.dma_start(
                    xn[:], x_d.ap()[tb0 * 128:(tb0 + 1) * 128, :])
                xn_pre[tb0] = xn
            ident_t = const.tile([128, 128], bf16)
            nc.sync.dma_start(ident_t[:], ident_d.ap())
            identf_t = const.tile([128, 128], f32)
            nc.sync.dma_start(identf_t[:], identf_d.ap())
            wq_t = const.tile([128, HB, NDC], bf16)
            wkv_t = const.tile([128, HB, 2 * D], bf16)
            for hb in range(HB):
                nc.sync.dma_start(
                    wq_t[:, hb, :],
                    wq_d.ap()[hb * 128:(hb + 1) * 128, :])
                nc.scalar.dma_start(
                    wkv_t[:, hb, :],
                    wkv_d.ap()[hb * 128:(hb + 1) * 128, :])
            masks_t = const.tile([128, len(MASK_DELTAS), 512], bf16)
            nc.scalar.dma_start(
                masks_t[:], masks_d.ap().rearrange("m p n -> p m n"))
            wo_t = const.tile([128, NDC // 128, H], bf16)
            nc.scalar.dma_start(
                wo_t[:], wo_d.ap().rearrange("(a p) n -> p a n", p=128))
            onesc_t = const.tile([128, 1], bf16)
            nc.vector.memset(onesc_t[:], 1.0)
            onesr_t = const.tile([1, 128], bf16)
            nc.vector.memset(onesr_t[:], 1.0)
            eps_t = const.tile([128, 1], f32)
            nc.vector.memset(eps_t[:], EPS)

            v_t = const.tile([128, TB, D], bf16)        # V, token-major
            kT_t = const.tile([128, 2, T], bf16)        # K^T, d-major
            aT_t = const.tile([128, NDC // 128, T], bf16)  # attn out^T

            x_ap = x_d.ap()

            qT_tiles = {}

            # ================= Phase 1: QKV projection =================
            def qkn_transposes(tb, qn, kn):
                # qn -> qT, kn -> kT  (PE transposes + ACT copies)
                qT = qT_tiles[tb // 4]
                for j in range(NDC // 128):
                    tp = pmm.tile([128, 128], bf16, name="tpq",
                                  tag="acc", space=PSUM)
                    nc.tensor.transpose(
                        tp[:], qn[:, j * 128:(j + 1) * 128], ident_t[:])
                    nc.scalar.copy(
                        qT[:, j, (tb % 4) * 128:(tb % 4 + 1) * 128], tp[:])
                for j in range(2):
                    tp = pmm.tile([128, 128], bf16, name="tpk",
                                  tag="acc", space=PSUM)
                    nc.tensor.transpose(
                        tp[:], kn[:, j * 128:(j + 1) * 128], ident_t[:])
                    nc.scalar.copy(
                        kT_t[:, j, tb * 128:(tb + 1) * 128], tp[:])

            # ---- attention / Wo stages (interleaved into the P1 loop as
            # soon as their K/V token blocks are available) ----
            def attn_head(qg, head):
                kb_lo = max(0, 4 * qg - 8)
                kb_hi = 4 * qg + 4
                qT = qT_tiles[qg]
                oTs = []
                for c in range(2):
                    oT = pacc.tile([128, 512], f32, name="oT", tag="acc")
                    oTs.append(oT)
                sums = pacc.tile([1, 512], f32, name="sums", tag="acc")
                for ki, kb in enumerate(range(kb_lo, kb_hi)):
                    st, sp = ki == 0, kb == kb_hi - 1
                    sT = pmm.tile([128, 512], f32, name="sT", tag="acc",
                                  space=PSUM)
                    for c in range(2):
                        nc.tensor.matmul(
                            sT[:],
                            kT_t[:, c, kb * 128:(kb + 1) * 128],
                            qT[:, head * 2 + c, :],
                            start=(c == 0), stop=(c == 1))
                    pT = pTp.tile([128, 512], bf16, name="pT")
                    nc.scalar.activation(pT[:], sT[:], AF.Exp,
                                         scale=float(D) ** -0.5)
                    delta = kb * 128 - qg * 512
                    if delta in mask_idx:
                        nc.vector.tensor_mul(
                            pT[:], pT[:], masks_t[:, mask_idx[delta], :])
                    for c in range(2):
                        nc.tensor.matmul(
                            oTs[c][:],
                            v_t[:, kb, c * 128:(c + 1) * 128],
                            pT[:], start=st, stop=sp)
                    nc.tensor.matmul(sums[:], onesc_t[:], pT[:],
                                     start=st, stop=sp)
                # normalize: att^T = oT / sums; the broadcast runs on
                # the idle GpSimd engine, off PE's critical path
                recip = smp.tile([1, 512], f32, name="recip")
                nc.vector.reciprocal(recip[:], sums[:])
                bcs = smp.tile([128, 512], f32, name="bcs")
                nc.gpsimd.partition_broadcast(bcs[:], recip[:])
                for c in range(2):
                    nc.vector.tensor_mul(
                        aT_t[:, head * 2 + c, qg * 512:(qg + 1) * 512],
                        oTs[c][:], bcs[:])

            def ns0(qg):
                return 4 // RS_SPLITS[qg]

            def wo_stage(qg):
                ypart = dramp.tile([512, H], bf16, name="ypart",
                                   tag=f"ypart{qg}")
                ns = RS_SPLITS[qg]
                rows = 512 // ns
                orows = rows // 4
                for part in range(ns):
                    for tl in range(part * ns0(qg), (part + 1) * ns0(qg)):
                        tb = qg * 4 + tl
                        yb = ybp.tile([128, H], bf16, name="yb")
                        for n in range(H // 512):
                            yp = pmm.tile([128, 512], f32, name="yp",
                                          tag="acc", space=PSUM)
                            for dch in range(NDC // 128):
                                nc.tensor.matmul(
                                    yp[:],
                                    aT_t[:, dch, tb * 128:(tb + 1) * 128],
                                    wo_t[:, dch, n * 512:(n + 1) * 512],
                                    start=(dch == 0),
                                    stop=(dch == NDC // 128 - 1))
                            nc.vector.tensor_copy(
                                yb[:, n * 512:(n + 1) * 512], yp[:])
                        nc.sync.dma_start(
                            ypart[tl * 128:(tl + 1) * 128, :], yb[:])
                    # fire this part's ReduceScatter as soon as its rows
                    # are in DRAM
                    yrs = dramp.tile([orows, H], bf16, name="yrs",
                                     tag=f"yrs{qg}_{part}")
                    nc.gpsimd.collective_compute(
                        "ReduceScatter", ALU.add,
                        replica_groups=[[0, 1, 2, 3], [4, 5, 6, 7]],
                        ins=[ypart[part * rows:(part + 1) * rows, :].opt()],
                        outs=[yrs[:].opt()])
                    nc.sync.dma_start(
                        y_outs[qg].ap()[part * orows:(part + 1) * orows, :],
                        yrs[:])

            # interleave points: after P1 iteration tb, emit the stages
            # whose inputs (K/V/Q^T blocks) are complete
            interleave = {
                4: [(attn_head, 0, 0)], 5: [(attn_head, 0, 1)],
                6: [(wo_stage, 0)],
                8: [(attn_head, 1, 0)], 9: [(attn_head, 1, 1)],
                10: [(wo_stage, 1)],
                12: [(attn_head, 2, 0)], 13: [(attn_head, 2, 1)],
                14: [(wo_stage, 2)],
            }

            pending_t = None   # (tb, qn, kn) whose transposes are deferred
            for tb in range(TB):
                qg = tb // 4
                if qg not in qT_tiles:
                    qT_tiles[qg] = qTp.tile([128, NDC // 128, 512], bf16,
                                            name="qT")
                if tb in xn_pre:
                    xn = xn_pre[tb]
                else:
                    xn = xnp.tile([128, H], bf16, name="xn")
                    nc.sync.dma_start(
                        xn[:], x_ap[tb * 128:(tb + 1) * 128, :])
                qp = pacc.tile([128, 512], f32, name="accq", tag="acc")
                kvp = pacc.tile([128, 512], f32, name="acck", tag="acc")
                # x^T chunks via PE transpose (fp32), 4 per PSUM bank, with
                # one wide cast-copy to bf16 per bank; batched ahead of the
                # matmuls so PE never waits on the copy round-trip
                xts = []
                for g4 in range(HB // 4):
                    tpx = pmm.tile([128, 512], bf16, name="tpx", tag="acc",
                                   space=PSUM)
                    for j in range(4):
                        nc.tensor.matmul(
                            tpx[:, j * 128:(j + 1) * 128],
                            xn[:, (g4 * 4 + j) * 128:(g4 * 4 + j + 1) * 128],
                            ident_t[:], is_transpose=True,
                            start=(j == 0), stop=(j == 3),
                            skip_group_check=True)
                    xt = xtp.tile([128, 512], bf16, name="xt")
                    if g4 % 2 == 0:
                        nc.scalar.copy(xt[:], tpx[:])
                    else:
                        nc.vector.tensor_copy(xt[:], tpx[:])
                    xts.append(xt)
                for hb in range(HB):
                    st, sp = hb == 0, hb == HB - 1
                    xt = xts[hb // 4][:, (hb % 4) * 128:(hb % 4 + 1) * 128]
                    nc.tensor.matmul(qp[:], xt, wq_t[:, hb, :],
                                     start=st, stop=sp)
                    nc.tensor.matmul(kvp[:], xt, wkv_t[:, hb, :],
                                     start=st, stop=sp)

                # previous block's qn/kn transposes: their rope inputs are
                # ready by now, so PE doesn't stall on the DVE chain
                if pending_t is not None:
                    qkn_transposes(*pending_t)

                # rope tables for this token block (one fused DMA)
                tabt = tabp.tile([128, len(tab_names), 128], f32,
                                 name="tabt")
                nc.sync.dma_start(
                    tabt[:], tabs_d.ap()[tb * 128:(tb + 1) * 128, :])
                tt = {name: tabt[:, ti, :]
                      for ti, name in enumerate(tab_names)}

                qn = qnp.tile([128, NDC], bf16, name="qn")
                kn = qnp.tile([128, D], bf16, name="kn")

                # rmsnorm stats for the 3 heads (2q + 1k), batched
                ssum3 = statp.tile([128, 3], f32, name="ssum3")
                for si, (src, col0) in enumerate(
                        [(qp, 0), (qp, D), (kvp, 0)]):
                    sqs = scrp.tile([128, 256], bf16, name="sqs")
                    nc.scalar.activation(sqs[:], src[:, col0:col0 + D],
                                         AF.Square,
                                         accum_out=ssum3[:, si:si + 1])
                rsq3 = statp.tile([128, 3], f32, name="rsq3")
                nc.scalar.activation(rsq3[:], ssum3[:], AF.Sqrt,
                                     scale=1.0 / D, bias=eps_t[:])
                r3 = statp.tile([128, 3], f32, name="r3")
                nc.vector.reciprocal(r3[:], rsq3[:])

                def rms_rope(src, col0, si, cn1, sn1, cn2, sn2, dst, dcol0):
                    r = r3[:, si:si + 1]
                    q1 = src[:, col0:col0 + 128]
                    q2 = src[:, col0 + 128:col0 + 256]
                    a1 = ropep.tile([128, 128], f32, name="a1")
                    b1 = ropep.tile([128, 128], f32, name="b1")
                    nc.vector.scalar_tensor_tensor(
                        a1[:], q1, r, tt[cn1], op0=ALU.mult, op1=ALU.mult)
                    nc.vector.scalar_tensor_tensor(
                        b1[:], q2, r, tt[sn1], op0=ALU.mult, op1=ALU.mult)
                    nc.vector.tensor_sub(
                        dst[:, dcol0:dcol0 + 128], a1[:], b1[:])
                    a2 = ropep.tile([128, 128], f32, name="a2")
                    b2 = ropep.tile([128, 128], f32, name="b2")
                    nc.vector.scalar_tensor_tensor(
                        a2[:], q2, r, tt[cn2], op0=ALU.mult, op1=ALU.mult)
                    nc.vector.scalar_tensor_tensor(
                        b2[:], q1, r, tt[sn2], op0=ALU.mult, op1=ALU.mult)
                    nc.vector.tensor_add(
                        dst[:, dcol0 + 128:dcol0 + 256], a2[:], b2[:])

                for h in range(NHC):
                    rms_rope(qp, h * D, h, "cq1", "sq1", "cq2", "sq2",
                             qn, h * D)
                rms_rope(kvp, 0, 2, "ck1", "sk1", "ck2", "sk2", kn, 0)
                # V: psum -> sbuf bf16
                nc.scalar.copy(v_t[:, tb, :], kvp[:, D:2 * D])
                pending_t = (tb, qn, kn)
                for stage in interleave.get(tb, ()):
                    stage[0](*stage[1:])
            qkn_transposes(*pending_t)

            # ====== tail: last query group's attention + Wo + RS ======
            attn_head(3, 0)
            attn_head(3, 1)
            wo_stage(3)

    nc.compile()
    _PROG_CACHE["prog"] = nc
    return nc


def _host_tables(segment_ids, cur_ind, q_scale, k_scale):
    """RoPE sin/cos tables with (1+scale) RMSNorm gains folded in."""
    segment_ids = np.asarray(segment_ids)
    ar = np.arange(T, dtype=np.int64)
    fraction = np.arange(0, D, 2, dtype=np.float64) / D
    freq = 1.0 / (ROPE_THETA ** fraction)
    out = []
    wq = 1.0 + np.asarray(q_scale, np.float64)
    wk = 1.0 + np.asarray(k_scale, np.float64)
    for b in range(B):
        row = segment_ids[b]
        nz = row != 0
        start = int(np.argmax(nz)) if nz.any() else 0
        pos = np.where(nz, ar - start, 2 ** 30).astype(np.float64)
        pos = pos + float(np.asarray(cur_ind))
        ang = pos[:, None] * freq[None, :]
        sin, cos = np.sin(ang), np.cos(ang)
        tabs = (
            cos * wq[None, :128], sin * wq[None, 128:],
            cos * wq[None, 128:], sin * wq[None, :128],
            cos * wk[None, :128], sin * wk[None, 128:],
            cos * wk[None, 128:], sin * wk[None, :128],
        )
        out.append(np.ascontiguousarray(
            np.concatenate(tabs, axis=1), np.float32))
    return out


def _host_masks():
    import ml_dtypes
    qq = np.arange(512)[None, :]
    kk = np.arange(128)[:, None]
    m = np.zeros((len(MASK_DELTAS), 128, 512), np.float32)
    for i, delta in enumerate(MASK_DELTAS):
        diff = qq - kk + (-delta)   # = q - k for q=qg*512+qq, k=kb*128+kk
        m[i] = ((diff >= 0) & (diff < WINDOW)).astype(np.float32)
    return m.astype(ml_dtypes.bfloat16)


def kernel(x, Wq, Wk, Wv, Wo, q_scale, k_scale, segment_ids, mask, cur_ind):
    global LAST_RESULTS
    _ensure_import_paths()
    _install_ntff_hook()
    import ml_dtypes
    from concourse import bass_utils

    bf = ml_dtypes.bfloat16
    nc = _build_program()

    x = np.asarray(x, np.float32).astype(bf)
    wq_b = np.asarray(Wq, np.float32).astype(bf)
    wk_b = np.asarray(Wk, np.float32).astype(bf)
    wv_b = np.asarray(Wv, np.float32).astype(bf)
    wo_b = np.asarray(Wo, np.float32).astype(bf)
    tabs = _host_tables(segment_ids, cur_ind, q_scale, k_scale)
    masks = _host_masks()
    ident = np.eye(128, dtype=np.float32).astype(bf)

    in_maps = []
    for core in range(N_CORES):
        b, g = divmod(core, NKV)
        im = {
            "x": x[b],
            "wq": np.ascontiguousarray(wq_b[:, g * NDC:(g + 1) * NDC]),
            "wkv": np.ascontiguousarray(np.concatenate(
                [wk_b[:, g * D:(g + 1) * D], wv_b[:, g * D:(g + 1) * D]],
                axis=1)),
            "wo": np.ascontiguousarray(wo_b[g * NDC:(g + 1) * NDC, :]),
            "masks": masks,
            "ident": ident,
            "identf": np.eye(128, dtype=np.float32),
            "tabs": tabs[b],
        }
        in_maps.append(im)

    res = bass_utils.run_bass_kernel_spmd(
        nc, in_maps, core_ids=list(range(N_CORES)))
    LAST_RESULTS = res

    y = np.empty((B, T, H), np.float32)
    for core in range(N_CORES):
        b, r = divmod(core, NKV)
        for qg in range(NQG):
            rows = res.results[core][f"y{qg}"].astype(np.float32)
            ns = RS_SPLITS[qg]
            prows = 512 // ns
            orows = prows // 4
            for part in range(ns):
                t0 = qg * 512 + part * prows + r * orows
                y[b, t0:t0 + orows, :] = \
                    rows[part * orows:(part + 1) * orows]
    return y


# revision 53
# speedup vs baseline: 1.1221x; 1.1221x over previous
"""Gemma3 sliding-window attention on 8 Trainium2 NeuronCores.

Sharding: 8 cores = 2 batches x 4 KV-head groups (2 query heads / 1 KV head
per core).  Per core: QKV projections (column-sharded), fused RMSNorm+RoPE
(q/k scales folded into host-built RoPE tables), sliding-window attention in
transposed score layout (S^T = K^T q), row-sharded Wo with a 4-way
ReduceScatter per 512-token group.  bf16 matmuls, fp32 PSUM accumulation.
"""

import os
import sys
import types

import numpy as np

B, T, H = 2, 2048, 2560
NH, NKV, D = 8, 4, 256
WINDOW = 1024
EPS = 1e-6
ROPE_THETA = 10000.0

N_CORES = 8
NHC = NH // NKV      # query heads per core (2)
NDC = NHC * D        # head dims per core (512)
TB = T // 128        # 16 token blocks
HB = H // 128        # 20 hidden chunks
NQG = T // 512       # 4 query groups
RS_SPLITS = [2, 2, 2, 2]   # ReduceScatter chunks per query group
# mask tile index by delta = kb*128 - qg*512
MASK_DELTAS = [0, 128, 256, 384, -640, -768, -896, -1024]

_TRN_ROOT_CANDIDATES = ("/opt/trn_rl_repo", "/root/.axon_site/_ro/trn_rl_repo")
_PROG_CACHE = {}
LAST_RESULTS = None


def _ensure_import_paths():
    for p in _TRN_ROOT_CANDIDATES:
        if os.path.isdir(p) and p not in sys.path:
            sys.path.insert(0, p)
            break


def _install_ntff_hook():
    """Register the axon NTFF profiling hook (missing antenv.axon_hooks shim)."""
    modname = "antenv.axon_hooks"
    if modname in sys.modules:
        return
    try:
        import antenv
        mod = types.ModuleType(modname)
        holder = {}
        mod.set_axon_ntff_profile_hook = lambda h: holder.__setitem__("h", h)
        mod.get_axon_ntff_profile_hook = lambda: holder.get("h")
        sys.modules[modname] = mod
        antenv.axon_hooks = mod
        from trn_agent_boot.trn_boot import _ntff_profile_via_ctypes
        mod.set_axon_ntff_profile_hook(
            _ntff_profile_via_ctypes("/opt/axon/libaxon_pjrt.so")
        )
    except Exception:
        pass


def _build_program():
    if "prog" in _PROG_CACHE:
        return _PROG_CACHE["prog"]

    _ensure_import_paths()
    import concourse.bass as bass
    import concourse.tile as tile
    from concourse import bacc, mybir

    dt = mybir.dt
    f32, bf16 = dt.float32, dt.bfloat16
    AF = mybir.ActivationFunctionType
    ALU = mybir.AluOpType
    PSUM = bass.MemorySpace.PSUM

    nc = bacc.Bacc("TRN2", target_bir_lowering=False, debug=False,
                   num_devices=N_CORES)

    # ---------------- I/O ----------------
    x_d = nc.dram_tensor("x", [T, H], bf16, kind="ExternalInput")
    wq_d = nc.dram_tensor("wq", [H, NDC], bf16, kind="ExternalInput")
    wkv_d = nc.dram_tensor("wkv", [H, 2 * D], bf16, kind="ExternalInput")
    wo_d = nc.dram_tensor("wo", [NDC, H], bf16, kind="ExternalInput")
    tab_names = ("cq1", "sq1", "cq2", "sq2", "ck1", "sk1", "ck2", "sk2")
    tabs_d = nc.dram_tensor("tabs", [T, len(tab_names) * (D // 2)], f32,
                            kind="ExternalInput")
    masks_d = nc.dram_tensor("masks", [len(MASK_DELTAS), 128, 512], bf16,
                             kind="ExternalInput")
    ident_d = nc.dram_tensor("ident", [128, 128], bf16, kind="ExternalInput")
    identf_d = nc.dram_tensor("identf", [128, 128], f32, kind="ExternalInput")
    y_outs = [nc.dram_tensor(f"y{j}", [128, H], bf16, kind="ExternalOutput")
              for j in range(NQG)]

    mask_idx = {d: i for i, d in enumerate(MASK_DELTAS)}

    with tile.TileContext(nc) as tc:
        import contextlib
        with contextlib.ExitStack() as ctx:
            const = ctx.enter_context(tc.tile_pool(name="const", bufs=1))
            # PSUM: tag "acc" (accumulators) 4 banks + tag "mm" (stream) 4
            pacc = ctx.enter_context(
                tc.tile_pool(name="pacc", bufs=8, space="PSUM"))
            pmm = pacc
            xnp = ctx.enter_context(tc.tile_pool(name="xnp", bufs=2))
            xtp = ctx.enter_context(tc.tile_pool(name="xtp", bufs=7))
            tabp = ctx.enter_context(tc.tile_pool(name="tabp", bufs=2))
            qnp = ctx.enter_context(tc.tile_pool(name="qnp", bufs=4))
            scrp = ctx.enter_context(tc.tile_pool(name="scrp", bufs=4))
            statp = ctx.enter_context(tc.tile_pool(name="statp", bufs=8))
            ropep = ctx.enter_context(tc.tile_pool(name="ropep", bufs=8))
            qTp = ctx.enter_context(tc.tile_pool(name="qTp", bufs=4))
            pTp = ctx.enter_context(tc.tile_pool(name="pTp", bufs=4))
            smp = ctx.enter_context(tc.tile_pool(name="smp", bufs=2))
            ybp = ctx.enter_context(tc.tile_pool(name="ybp", bufs=2))
            dramp = ctx.enter_context(
                tc.tile_pool(name="dramp", bufs=1, space="DRAM"))

            # -------- resident constants --------
            # first x blocks before the bulky weight loads so PE can start
            xn_pre = {}
            for tb0 in range(2):
                xn = xnp.tile([128, H], bf16, name="xn")
                nc.sync.dma_start(
                    xn[:], x_d.ap()[tb0 * 128:(tb0 + 1) * 128, :])
                xn_pre[tb0] = xn
            ident_t = const.tile([128, 128], bf16)
            nc.sync.dma_start(ident_t[:], ident_d.ap())
            identf_t = const.tile([128, 128], f32)
            nc.sync.dma_start(identf_t[:], identf_d.ap())
            wq_t = const.tile([128, HB, NDC], bf16)
            wkv_t = const.tile([128, HB, 2 * D], bf16)
            for hb in range(HB):
                nc.gpsimd.dma_start(
                    wq_t[:, hb, :],
                    wq_d.ap()[hb * 128:(hb + 1) * 128, :])
                nc.gpsimd.dma_start(
                    wkv_t[:, hb, :],
                    wkv_d.ap()[hb * 128:(hb + 1) * 128, :])
            masks_t = const.tile([128, len(MASK_DELTAS), 512], bf16)
            nc.gpsimd.dma_start(
                masks_t[:], masks_d.ap().rearrange("m p n -> p m n"))
            wo_t = const.tile([128, NDC // 128, H], bf16)
            nc.gpsimd.dma_start(
                wo_t[:], wo_d.ap().rearrange("(a p) n -> p a n", p=128))
            onesc_t = const.tile([128, 1], bf16)
            nc.vector.memset(onesc_t[:], 1.0)
            onesr_t = const.tile([1, 128], bf16)
            nc.vector.memset(onesr_t[:], 1.0)
            eps_t = const.tile([128, 1], f32)
            nc.vector.memset(eps_t[:], EPS)

            v_t = const.tile([128, TB, D], bf16)        # V, token-major
            kT_t = const.tile([128, 2, T], bf16)        # K^T, d-major
            aT_t = const.tile([128, NDC // 128, T], bf16)  # attn out^T

            x_ap = x_d.ap()

            qT_tiles = {}

            # ================= Phase 1: QKV projection =================
            def qkn_transposes(tb, qn, kn):
                # qn -> qT, kn -> kT  (PE transposes + ACT copies)
                qT = qT_tiles[tb // 4]
                for j in range(NDC // 128):
                    tp = pmm.tile([128, 128], bf16, name="tpq",
                                  tag="acc", space=PSUM)
                    nc.tensor.transpose(
                        tp[:], qn[:, j * 128:(j + 1) * 128], ident_t[:])
                    nc.scalar.copy(
                        qT[:, j, (tb % 4) * 128:(tb % 4 + 1) * 128], tp[:])
                for j in range(2):
                    tp = pmm.tile([128, 128], bf16, name="tpk",
                                  tag="acc", space=PSUM)
                    nc.tensor.transpose(
                        tp[:], kn[:, j * 128:(j + 1) * 128], ident_t[:])
                    nc.scalar.copy(
                        kT_t[:, j, tb * 128:(tb + 1) * 128], tp[:])

            # ---- attention / Wo stages (interleaved into the P1 loop as
            # soon as their K/V token blocks are available) ----
            def attn_head(qg, head):
                kb_lo = max(0, 4 * qg - 8)
                kb_hi = 4 * qg + 4
                qT = qT_tiles[qg]
                oTs = []
                for c in range(2):
                    oT = pacc.tile([128, 512], f32, name="oT", tag="acc")
                    oTs.append(oT)
                sums = pacc.tile([1, 512], f32, name="sums", tag="acc")
                for ki, kb in enumerate(range(kb_lo, kb_hi)):
                    st, sp = ki == 0, kb == kb_hi - 1
                    sT = pmm.tile([128, 512], f32, name="sT", tag="acc",
                                  space=PSUM)
                    for c in range(2):
                        nc.tensor.matmul(
                            sT[:],
                            kT_t[:, c, kb * 128:(kb + 1) * 128],
                            qT[:, head * 2 + c, :],
                            start=(c == 0), stop=(c == 1))
                    pT = pTp.tile([128, 512], bf16, name="pT")
                    nc.scalar.activation(pT[:], sT[:], AF.Exp,
                                         scale=float(D) ** -0.5)
                    delta = kb * 128 - qg * 512
                    if delta in mask_idx:
                        nc.vector.tensor_mul(
                            pT[:], pT[:], masks_t[:, mask_idx[delta], :])
                    for c in range(2):
                        nc.tensor.matmul(
                            oTs[c][:],
                            v_t[:, kb, c * 128:(c + 1) * 128],
                            pT[:], start=st, stop=sp)
                    nc.tensor.matmul(sums[:], onesc_t[:], pT[:],
                                     start=st, stop=sp)
                # normalize: att^T = oT / sums; the broadcast runs on
                # the idle GpSimd engine, off PE's critical path
                recip = smp.tile([1, 512], f32, name="recip")
                nc.vector.reciprocal(recip[:], sums[:])
                bcs = smp.tile([128, 512], f32, name="bcs")
                nc.gpsimd.partition_broadcast(bcs[:], recip[:])
                for c in range(2):
                    nc.vector.tensor_mul(
                        aT_t[:, head * 2 + c, qg * 512:(qg + 1) * 512],
                        oTs[c][:], bcs[:])

            def ns0(qg):
                return 4 // RS_SPLITS[qg]

            def wo_stage(qg):
                ypart = dramp.tile([512, H], bf16, name="ypart",
                                   tag=f"ypart{qg}")
                ns = RS_SPLITS[qg]
                rows = 512 // ns
                orows = rows // 4
                for part in range(ns):
                    for tl in range(part * ns0(qg), (part + 1) * ns0(qg)):
                        tb = qg * 4 + tl
                        yb = ybp.tile([128, H], bf16, name="yb")
                        for n in range(H // 512):
                            yp = pmm.tile([128, 512], f32, name="yp",
                                          tag="acc", space=PSUM)
                            for dch in range(NDC // 128):
                                nc.tensor.matmul(
                                    yp[:],
                                    aT_t[:, dch, tb * 128:(tb + 1) * 128],
                                    wo_t[:, dch, n * 512:(n + 1) * 512],
                                    start=(dch == 0),
                                    stop=(dch == NDC // 128 - 1))
                            nc.vector.tensor_copy(
                                yb[:, n * 512:(n + 1) * 512], yp[:])
                        nc.sync.dma_start(
                            ypart[tl * 128:(tl + 1) * 128, :], yb[:])
                    # fire this part's ReduceScatter as soon as its rows
                    # are in DRAM
                    yrs = dramp.tile([orows, H], bf16, name="yrs",
                                     tag=f"yrs{qg}_{part}")
                    nc.gpsimd.collective_compute(
                        "ReduceScatter", ALU.add,
                        replica_groups=[[0, 1, 2, 3], [4, 5, 6, 7]],
                        ins=[ypart[part * rows:(part + 1) * rows, :].opt()],
                        outs=[yrs[:].opt()])
                    nc.sync.dma_start(
                        y_outs[qg].ap()[part * orows:(part + 1) * orows, :],
                        yrs[:])

            # interleave points: after P1 iteration tb, emit the stages
            # whose inputs (K/V/Q^T blocks) are complete
            interleave = {
                4: [(attn_head, 0, 0)], 5: [(attn_head, 0, 1)],
                6: [(wo_stage, 0)],
                8: [(attn_head, 1, 0)], 9: [(attn_head, 1, 1)],
                10: [(wo_stage, 1)],
                12: [(attn_head, 2, 0)], 13: [(attn_head, 2, 1)],
                14: [(wo_stage, 2)],
            }

            pending_t = None   # (tb, qn, kn) whose transposes are deferred
            for tb in range(TB):
                qg = tb // 4
                if qg not in qT_tiles:
                    qT_tiles[qg] = qTp.tile([128, NDC // 128, 512], bf16,
                                            name="qT")
                if tb in xn_pre:
                    xn = xn_pre[tb]
                else:
                    xn = xnp.tile([128, H], bf16, name="xn")
                    nc.sync.dma_start(
                        xn[:], x_ap[tb * 128:(tb + 1) * 128, :])
                qp = pacc.tile([128, 512], f32, name="accq", tag="acc")
                kvp = pacc.tile([128, 512], f32, name="acck", tag="acc")
                # x^T chunks via PE transpose (fp32), 4 per PSUM bank, with
                # one wide cast-copy to bf16 per bank; batched ahead of the
                # matmuls so PE never waits on the copy round-trip
                xts = []
                for g4 in range(HB // 4):
                    tpx = pmm.tile([128, 512], bf16, name="tpx", tag="acc",
                                   space=PSUM)
                    for j in range(4):
                        nc.tensor.matmul(
                            tpx[:, j * 128:(j + 1) * 128],
                            xn[:, (g4 * 4 + j) * 128:(g4 * 4 + j + 1) * 128],
                            ident_t[:], is_transpose=True,
                            start=(j == 0), stop=(j == 3),
                            skip_group_check=True)
                    xt = xtp.tile([128, 512], bf16, name="xt")
                    if g4 % 2 == 0:
                        nc.scalar.copy(xt[:], tpx[:])
                    else:
                        nc.vector.tensor_copy(xt[:], tpx[:])
                    xts.append(xt)
                for hb in range(HB):
                    st, sp = hb == 0, hb == HB - 1
                    xt = xts[hb // 4][:, (hb % 4) * 128:(hb % 4 + 1) * 128]
                    nc.tensor.matmul(qp[:], xt, wq_t[:, hb, :],
                                     start=st, stop=sp)
                    nc.tensor.matmul(kvp[:], xt, wkv_t[:, hb, :],
                                     start=st, stop=sp)

                # previous block's qn/kn transposes: their rope inputs are
                # ready by now, so PE doesn't stall on the DVE chain
                if pending_t is not None:
                    qkn_transposes(*pending_t)

                # rope tables for this token block (one fused DMA)
                tabt = tabp.tile([128, len(tab_names), 128], f32,
                                 name="tabt")
                nc.sync.dma_start(
                    tabt[:], tabs_d.ap()[tb * 128:(tb + 1) * 128, :])
                tt = {name: tabt[:, ti, :]
                      for ti, name in enumerate(tab_names)}

                qn = qnp.tile([128, NDC], bf16, name="qn")
                kn = qnp.tile([128, D], bf16, name="kn")

                # rmsnorm stats for the 3 heads (2q + 1k), batched
                ssum3 = statp.tile([128, 3], f32, name="ssum3")
                for si, (src, col0) in enumerate(
                        [(qp, 0), (qp, D), (kvp, 0)]):
                    sqs = scrp.tile([128, 256], bf16, name="sqs")
                    nc.scalar.activation(sqs[:], src[:, col0:col0 + D],
                                         AF.Square,
                                         accum_out=ssum3[:, si:si + 1])
                rsq3 = statp.tile([128, 3], f32, name="rsq3")
                nc.scalar.activation(rsq3[:], ssum3[:], AF.Sqrt,
                                     scale=1.0 / D, bias=eps_t[:])
                r3 = statp.tile([128, 3], f32, name="r3")
                nc.vector.reciprocal(r3[:], rsq3[:])

                def rms_rope(src, col0, si, cn1, sn1, cn2, sn2, dst, dcol0):
                    r = r3[:, si:si + 1]
                    q1 = src[:, col0:col0 + 128]
                    q2 = src[:, col0 + 128:col0 + 256]
                    a1 = ropep.tile([128, 128], f32, name="a1")
                    b1 = ropep.tile([128, 128], f32, name="b1")
                    nc.vector.scalar_tensor_tensor(
                        a1[:], q1, r, tt[cn1], op0=ALU.mult, op1=ALU.mult)
                    nc.vector.scalar_tensor_tensor(
                        b1[:], q2, r, tt[sn1], op0=ALU.mult, op1=ALU.mult)
                    nc.vector.tensor_sub(
                        dst[:, dcol0:dcol0 + 128], a1[:], b1[:])
                    a2 = ropep.tile([128, 128], f32, name="a2")
                    b2 = ropep.tile([128, 128], f32, name="b2")
                    nc.vector.scalar_tensor_tensor(
                        a2[:], q2, r, tt[cn2], op0=ALU.mult, op1=ALU.mult)
                    nc.vector.scalar_tensor_tensor(
                        b2[:], q1, r, tt[sn2], op0=ALU.mult, op1=ALU.mult)
                    nc.vector.tensor_add(
                        dst[:, dcol0 + 128:dcol0 + 256], a2[:], b2[:])

                for h in range(NHC):
                    rms_rope(qp, h * D, h, "cq1", "sq1", "cq2", "sq2",
                             qn, h * D)
                rms_rope(kvp, 0, 2, "ck1", "sk1", "ck2", "sk2", kn, 0)
                # V: psum -> sbuf bf16
                nc.scalar.copy(v_t[:, tb, :], kvp[:, D:2 * D])
                pending_t = (tb, qn, kn)
                for stage in interleave.get(tb, ()):
                    stage[0](*stage[1:])
            qkn_transposes(*pending_t)

            # ====== tail: last query group's attention + Wo + RS ======
            attn_head(3, 0)
            attn_head(3, 1)
            wo_stage(3)

    nc.compile()
    _PROG_CACHE["prog"] = nc
    return nc


def _host_tables(segment_ids, cur_ind, q_scale, k_scale):
    """RoPE sin/cos tables with (1+scale) RMSNorm gains folded in."""
    segment_ids = np.asarray(segment_ids)
    ar = np.arange(T, dtype=np.int64)
    fraction = np.arange(0, D, 2, dtype=np.float64) / D
    freq = 1.0 / (ROPE_THETA ** fraction)
    out = []
    wq = 1.0 + np.asarray(q_scale, np.float64)
    wk = 1.0 + np.asarray(k_scale, np.float64)
    for b in range(B):
        row = segment_ids[b]
        nz = row != 0
        start = int(np.argmax(nz)) if nz.any() else 0
        pos = np.where(nz, ar - start, 2 ** 30).astype(np.float64)
        pos = pos + float(np.asarray(cur_ind))
        ang = pos[:, None] * freq[None, :]
        sin, cos = np.sin(ang), np.cos(ang)
        tabs = (
            cos * wq[None, :128], sin * wq[None, 128:],
            cos * wq[None, 128:], sin * wq[None, :128],
            cos * wk[None, :128], sin * wk[None, 128:],
            cos * wk[None, 128:], sin * wk[None, :128],
        )
        out.append(np.ascontiguousarray(
            np.concatenate(tabs, axis=1), np.float32))
    return out


def _host_masks():
    import ml_dtypes
    qq = np.arange(512)[None, :]
    kk = np.arange(128)[:, None]
    m = np.zeros((len(MASK_DELTAS), 128, 512), np.float32)
    for i, delta in enumerate(MASK_DELTAS):
        diff = qq - kk + (-delta)   # = q - k for q=qg*512+qq, k=kb*128+kk
        m[i] = ((diff >= 0) & (diff < WINDOW)).astype(np.float32)
    return m.astype(ml_dtypes.bfloat16)


def kernel(x, Wq, Wk, Wv, Wo, q_scale, k_scale, segment_ids, mask, cur_ind):
    global LAST_RESULTS
    _ensure_import_paths()
    _install_ntff_hook()
    import ml_dtypes
    from concourse import bass_utils

    bf = ml_dtypes.bfloat16
    nc = _build_program()

    x = np.asarray(x, np.float32).astype(bf)
    wq_b = np.asarray(Wq, np.float32).astype(bf)
    wk_b = np.asarray(Wk, np.float32).astype(bf)
    wv_b = np.asarray(Wv, np.float32).astype(bf)
    wo_b = np.asarray(Wo, np.float32).astype(bf)
    tabs = _host_tables(segment_ids, cur_ind, q_scale, k_scale)
    masks = _host_masks()
    ident = np.eye(128, dtype=np.float32).astype(bf)

    in_maps = []
    for core in range(N_CORES):
        b, g = divmod(core, NKV)
        im = {
            "x": x[b],
            "wq": np.ascontiguousarray(wq_b[:, g * NDC:(g + 1) * NDC]),
            "wkv": np.ascontiguousarray(np.concatenate(
                [wk_b[:, g * D:(g + 1) * D], wv_b[:, g * D:(g + 1) * D]],
                axis=1)),
            "wo": np.ascontiguousarray(wo_b[g * NDC:(g + 1) * NDC, :]),
            "masks": masks,
            "ident": ident,
            "identf": np.eye(128, dtype=np.float32),
            "tabs": tabs[b],
        }
        in_maps.append(im)

    res = bass_utils.run_bass_kernel_spmd(
        nc, in_maps, core_ids=list(range(N_CORES)))
    LAST_RESULTS = res

    y = np.empty((B, T, H), np.float32)
    for core in range(N_CORES):
        b, r = divmod(core, NKV)
        for qg in range(NQG):
            rows = res.results[core][f"y{qg}"].astype(np.float32)
            ns = RS_SPLITS[qg]
            prows = 512 // ns
            orows = prows // 4
            for part in range(ns):
                t0 = qg * 512 + part * prows + r * orows
                y[b, t0:t0 + orows, :] = \
                    rows[part * orows:(part + 1) * orows]
    return y
